# revision 2
# baseline (speedup 1.0000x reference)
"""Multi-head causal attention (B=4, S=2048, D=1024, H=16) on 8 trn2 cores. v3.

Sharding: (batch x head-group) grid -> core c handles batch c//2, heads
[8*(c%2), 8*(c%2)+8).  Host sums the two partial outputs per batch + bo.

v3: fully interleaved single-phase schedule.  The attention mk-loops consume
"filler" thunks (qkv projection groups, then output-projection tiles) so the
PE stream stays dense while ACT churns exp -- this keeps the HAM clock gate
at 8/8 (v2 lost ~150us to half-clock PE during its attention phase).

  - everything bf16 (x, Wq/k/v/o, q/k/v, exp, att) except PSUM f32 and the
    fp32 normalize chain; end-to-end emulated rel err 3.0e-3 vs 2e-2 budget.
  - exp fused per (hl, mk) over both 512-q blocks: one [128, <=1024] ACT op
    reading a 2-bank PSUM scores tile (halves ACT instruction overhead).
  - scores software-pipelined one mk ahead of exp/attended.
  - softmax normalize: PSUM drained per block by one DVE copy (rows 0..64);
    reciprocal bounce batched per (hp, half), muls off critical path.
  - schedule: [qk m0, v] -> half-0 attention hp0..3 (fillers: rest of qkv)
    -> half-1 attention hp0..3 (fillers: outproj st0..7) -> outproj st8..15.
"""

import sys

import numpy as np

sys.path.insert(0, "/opt/trn_rl_repo")

from collections import deque
from contextlib import ExitStack

import bass_rust

import concourse.bass as bass
import concourse.mybir as mybir
import concourse.tile as tile

_MAXW = 1


def _sem_ranges(nums):
    nums = sorted(nums)
    out = []
    start = prev = nums[0]
    for n in nums[1:]:
        if n == prev + 1:
            prev = n
            continue
        out.append(range(start, prev + 1))
        start = prev = n
    out.append(range(start, prev + 1))
    return out


def _install_compat():
    if getattr(bass, "_mha_compat_installed", False):
        return
    bass._mha_compat_installed = True
    from concourse.bass import SemaphoreHandle

    def clear_and_free_semaphores(self, sems):
        if not sems:
            return
        sem_nums = [s.num if isinstance(s, SemaphoreHandle) else s for s in sems]
        for r in _sem_ranges(sem_nums):
            assert self._state.free_isdisjoint(r)
            self.gpsimd.dma_reset(r)
        self._state.prepend_free_semaphores(sem_nums)
        for poison_set in self._tile_sem_poison_stack:
            poison_set.update(sem_nums)

    bass.Bass.clear_and_free_semaphores = clear_and_free_semaphores


def _split_sync_waits(nc):
    def new_nop(engine):
        binst = nc.engines[engine].isa(
            nc.isa.Opcode.NEURON_ISA_TPB_OPCODE_NOP, {}
        )
        inst = binst.ins
        bb = nc.cur_bb.bb
        assert bb.instructions and bb.instructions[-1] is inst
        bb.instructions.pop()
        return inst

    for func in nc.m.functions:
        for blk in func.blocks:
            snapshot = list(blk.instructions)
            if not any(
                i.sync_info and i.sync_info.on_wait and len(i.sync_info.on_wait) > _MAXW
                for i in snapshot
            ):
                continue
            new = []
            for inst in snapshot:
                si = inst.sync_info
                waits = list(si.on_wait) if si and si.on_wait else []
                if len(waits) > _MAXW:
                    for w in waits[:-_MAXW]:
                        nop = new_nop(inst.engine)
                        nop.sync_info = bass_rust.SyncInfo(on_wait=[w], on_update=[])
                        new.append(nop)
                    upd = list(si.on_update) if si and si.on_update else []
                    inst.sync_info = bass_rust.SyncInfo(
                        on_wait=waits[-_MAXW:], on_update=upd
                    )
                new.append(inst)
            blk.instructions[:] = new


P = 128
S = 2048
D = 1024
EL = 512
NH = 8
DH = 64
NCORES = 8
SCALE = 1.0 / 8.0
NEG = -1.0e30

ST = S // P
DT = D // P
ET = EL // P
HQ = 1024

F32 = mybir.dt.float32
F32R = mybir.dt.float32r
BF16 = mybir.dt.bfloat16

_PROGRAM_CACHE = {}


def build_program(mode, split_waits=True):
    assert mode in ("causal", "full")
    _install_compat()
    nc = bass.Bass("TRN2", target_bir_lowering=False, debug=False)

    xt_d = nc.dram_tensor("xt", [D, S], BF16, kind="ExternalInput").ap()
    wqt_d = nc.dram_tensor("wqt", [D, EL], BF16, kind="ExternalInput").ap()
    wkt_d = nc.dram_tensor("wkt", [D, EL], BF16, kind="ExternalInput").ap()
    wvt_d = nc.dram_tensor("wvt", [D, EL], BF16, kind="ExternalInput").ap()
    wot_d = nc.dram_tensor("wot", [EL, D], BF16, kind="ExternalInput").ap()
    bq_d = nc.dram_tensor("bq", [EL, 1], F32, kind="ExternalInput").ap()
    bvrep_d = nc.dram_tensor("bvrep", [P, EL], F32, kind="ExternalInput").ap()
    tri_d = nc.dram_tensor("tri", [P, P], F32, kind="ExternalInput").ap()
    trib_d = nc.dram_tensor("trib", [P, P], BF16, kind="ExternalInput").ap()
    out_d = nc.dram_tensor("out", [S, D], F32, kind="ExternalOutput").ap()

    causal = mode == "causal"
    Exp = mybir.ActivationFunctionType.Exp

    with ExitStack() as ctx:
        tc = ctx.enter_context(tile.TileContext(nc))
        consts = ctx.enter_context(tc.tile_pool(name="consts", bufs=1))
        wpool = ctx.enter_context(tc.tile_pool(name="w", bufs=1))
        xpool = ctx.enter_context(tc.tile_pool(name="x", bufs=1))
        qkvp = ctx.enter_context(tc.tile_pool(name="qkv", bufs=1))
        attp = ctx.enter_context(tc.tile_pool(name="attsb", bufs=1))
        attu_p = ctx.enter_context(tc.tile_pool(name="attun", bufs=1))
        expp = ctx.enter_context(tc.tile_pool(name="exp", bufs=1))
        denp_pool = ctx.enter_context(tc.tile_pool(name="den", bufs=1))
        repp = ctx.enter_context(tc.tile_pool(name="rep", bufs=1))
        dramp = ctx.enter_context(tc.tile_pool(name="dram", bufs=2, space="DRAM"))
        outp = ctx.enter_context(tc.tile_pool(name="outsb", bufs=3))
        psum = ctx.enter_context(tc.tile_pool(name="ps", bufs=1, space="PSUM"))

        # ---- constants ----
        bq_sb = consts.tile([P, ET], F32)
        for m in range(ET):
            nc.sync.dma_start(bq_sb[:, m : m + 1], bq_d[m * P : (m + 1) * P, :])
        bvrep_sb = consts.tile([P, EL], F32)
        nc.sync.dma_start(bvrep_sb[:], bvrep_d)
        if causal:
            trib_sb = consts.tile([P, P], BF16)
            nc.sync.dma_start(trib_sb[:], trib_d)
        # dummy exp: pull ACT_TABLE_LOAD into the DMA-bound start
        dummy = consts.tile([1, 1], F32)
        nc.scalar.activation(dummy[:], bvrep_sb[0:1, 0:1], Exp, scale=0.0)

        # ---- weights + x: DMA order = wq, wk, xt c0, wv, xt c1, c2, c3, wot ----
        wq_sb = [wpool.tile([P, EL], BF16, tag=f"wq{k}", name=f"wq{k}") for k in range(DT)]
        wk_sb = [wpool.tile([P, EL], BF16, tag=f"wk{k}", name=f"wk{k}") for k in range(DT)]
        wv_sb = [wpool.tile([P, EL], BF16, tag=f"wv{k}", name=f"wv{k}") for k in range(DT)]
        wot_sb = [wpool.tile([P, D], BF16, tag=f"wo{kt}", name=f"wo{kt}") for kt in range(ET)]
        xt_sb = [xpool.tile([P, S], BF16, tag=f"xt{k}", name=f"xt{k}") for k in range(DT)]
        for k in range(DT):
            nc.sync.dma_start(wq_sb[k][:], wqt_d[k * P : (k + 1) * P, :])
            nc.sync.dma_start(xt_sb[k][:, 0:512], xt_d[k * P : (k + 1) * P, 0:512])
        for k in range(DT):
            nc.sync.dma_start(wk_sb[k][:], wkt_d[k * P : (k + 1) * P, :])
        for k in range(DT):
            nc.sync.dma_start(wv_sb[k][:], wvt_d[k * P : (k + 1) * P, :])
        for c in range(1, 4):
            s0 = c * 512
            for k in range(DT):
                nc.sync.dma_start(
                    xt_sb[k][:, s0 : s0 + 512], xt_d[k * P : (k + 1) * P, s0 : s0 + 512]
                )
        for kt in range(ET):
            nc.sync.dma_start(wot_sb[kt][:], wot_d[kt * P : (kt + 1) * P, :])

        # ---- persistent qkv / attention outputs ----
        qt_sb = [qkvp.tile([P, S], BF16, tag=f"qt{m}", name=f"qt{m}") for m in range(ET)]
        kt_sb = [qkvp.tile([P, S], BF16, tag=f"kt{m}", name=f"kt{m}") for m in range(ET)]
        v_sb = [qkvp.tile([P, NH * (DH + 1)], BF16, tag=f"v{st}", name=f"vsb{st}") for st in range(ST)]
        att_sb = [attp.tile([P, S], BF16, tag=f"att{kt}", name=f"attsb{kt}") for kt in range(ET)]

        # ---- emitters ----
        def qk_group(m, c):
            s0 = c * 512
            pq = psum.tile([P, 512], F32, tag="pqkv", bufs=2, name=f"pq{m}{c}")
            for k in range(DT):
                nc.tensor.matmul(
                    pq[:],
                    wq_sb[k][:, m * P : (m + 1) * P],
                    xt_sb[k][:, s0 : s0 + 512],
                    start=(k == 0),
                    stop=(k == DT - 1),
                )
            nc.vector.tensor_scalar_add(
                qt_sb[m][:, s0 : s0 + 512], pq[:], bq_sb[:, m : m + 1]
            )
            pk = psum.tile([P, 512], F32, tag="pqkv", bufs=2, name=f"pk{m}{c}")
            for k in range(DT):
                nc.tensor.matmul(
                    pk[:],
                    wk_sb[k][:, m * P : (m + 1) * P],
                    xt_sb[k][:, s0 : s0 + 512],
                    start=(k == 0),
                    stop=(k == DT - 1),
                )
            nc.vector.tensor_copy(kt_sb[m][:, s0 : s0 + 512], pk[:])

        def v_group(st):
            pv = psum.tile([P, EL], F32, tag="pqkv", bufs=2, name=f"pv{st}")
            for k in range(DT):
                nc.tensor.matmul(
                    pv[:],
                    xt_sb[k][:, st * P : (st + 1) * P],
                    wv_sb[k][:],
                    start=(k == 0),
                    stop=(k == DT - 1),
                )
            vdst = v_sb[st][:].rearrange("p (h c) -> p h c", c=DH + 1)
            nc.vector.tensor_add(
                vdst[:, :, 0:DH],
                pv[:].rearrange("p (h c) -> p h c", c=DH),
                bvrep_sb[:].rearrange("p (h c) -> p h c", c=DH),
            )
            nc.gpsimd.memset(vdst[:, :, DH : DH + 1], 1.0)

        def outproj_st(st):
            ot = outp.tile([P, D], F32, tag="out", name="ot")
            for eb in range(2):
                po = psum.tile([P, 512], F32, tag="pqkv", bufs=2, name=f"po{st}{eb}")
                for kt in range(ET):
                    nc.tensor.matmul(
                        po[:],
                        att_sb[kt][:, st * P : (st + 1) * P],
                        wot_sb[kt][:, eb * 512 : eb * 512 + 512],
                        start=(kt == 0),
                        stop=(kt == ET - 1),
                    )
                nc.vector.tensor_copy(ot[:, eb * 512 : eb * 512 + 512], po[:])
            nc.sync.dma_start(out_d[st * P : (st + 1) * P, :], ot[:])

        def attn_pass(hp, half, hl, attu, fillers, every):
            """One (head, q-half) pass: mk loop, scores pipelined 1 ahead."""
            q0 = half * HQ
            r0 = hl * DH
            h = 2 * hp + hl
            mk_hi = (half + 1) * 8 if causal else ST
            att_ps = {}
            for qbl in range(2):
                att_ps[qbl] = psum.tile(
                    [P, 512], F32, tag="att", bufs=2, name=f"at{half}{hp}{hl}{qbl}"
                )

            def finish(mk, sp, c0p):
                k0 = mk * P
                ex = expp.tile([P, 2 * 512], BF16, tag="exp", bufs=6, name="ex")
                nc.scalar.activation(
                    ex[:, c0p : 2 * 512], sp[:, c0p : 2 * 512], Exp, scale=SCALE
                )
                if causal and k0 >= q0:
                    # diag 0/1 mask post-exp on the idle gpsimd engine (keeps
                    # the DVE queue off the scores->exp critical path)
                    nc.gpsimd.tensor_mul(
                        ex[:, c0p : c0p + P], ex[:, c0p : c0p + P], trib_sb[:]
                    )
                for qbl in range(2):
                    qb0 = q0 + qbl * 512
                    if causal and qb0 + 512 <= k0:
                        continue
                    c0b = max(0, k0 - qb0) if causal else 0
                    lo = qbl * 512 + c0b
                    stop_mk = (
                        min((qb0 + 512) // P - 1, mk_hi - 1) if causal else ST - 1
                    )
                    nc.tensor.matmul(
                        att_ps[qbl][0 : DH + 1, c0b:512],
                        v_sb[mk][:, h * (DH + 1) : (h + 1) * (DH + 1)],
                        ex[:, lo : (qbl + 1) * 512],
                        start=(mk == 0),
                        stop=(mk == stop_mk),
                        skip_group_check=True,
                    )
                    if mk == stop_mk:
                        blk = qbl * 2 + hl
                        nc.vector.tensor_copy(
                            attu[0 : DH + 1, blk * 512 : (blk + 1) * 512],
                            att_ps[qbl][0 : DH + 1, :],
                        )

            pend = None
            for mk in range(mk_hi):
                k0 = mk * P
                c0p = max(0, k0 - q0) if causal else 0
                sp = psum.tile([P, 2 * 512], F32, tag="sc", bufs=2, name="sc")
                for qbl in range(2):
                    qb0 = q0 + qbl * 512
                    if causal and qb0 + 512 <= k0:
                        continue
                    c0b = max(0, k0 - qb0) if causal else 0
                    lo = qbl * 512 + c0b
                    nc.tensor.matmul(
                        sp[:, lo : (qbl + 1) * 512],
                        kt_sb[hp][r0 : r0 + DH, k0 : k0 + P],
                        qt_sb[hp][r0 : r0 + DH, q0 + lo : q0 + (qbl + 1) * 512],
                        start=True,
                        stop=True,
                    )
                if pend is not None:
                    finish(*pend)
                pend = (mk, sp, c0p)
                if fillers and mk % every == every - 1:
                    fillers.popleft()()
            finish(*pend)

        def normalize_qbl(hp, half, attu, qbl):
            # per-qbl chain: starts as soon as both hl drains of this qbl are
            # in, so the last region's outproj tail only waits ~one chain
            q0 = half * HQ
            o = qbl * 2 * 512
            dend = dramp.tile([1, 2 * 512], F32, tag="dend", name="dend")
            nc.sync.dma_start(dend[:], attu[DH : DH + 1, o : o + 1024])
            dpt = denp_pool.tile([P, 8], F32, tag="denp", bufs=2, name="dpt")
            nc.sync.dma_start(dpt[:], dend[:].rearrange("o (p c) -> (o p) c", c=8))
            nc.vector.reciprocal(dpt[:], dpt[:])
            dend2 = dramp.tile([1, 2 * 512], F32, tag="dend2", name="dend2")
            nc.sync.dma_start(dend2[:].rearrange("o (p c) -> (o p) c", c=8), dpt[:])
            rep = repp.tile([DH, 2 * 512], F32, tag="rep", bufs=3, name="rep")
            nc.sync.dma_start(rep[:], dend2[:].broadcast_to([DH, 2 * 512]))
            for hl in range(2):
                nc.vector.tensor_mul(
                    att_sb[hp][
                        hl * DH : (hl + 1) * DH,
                        q0 + qbl * 512 : q0 + qbl * 512 + 512,
                    ],
                    attu[0:DH, o + hl * 512 : o + hl * 512 + 512],
                    rep[:, hl * 512 : hl * 512 + 512],
                )

        def normalize(hp, half, attu):
            normalize_qbl(hp, half, attu, 0)
            normalize_qbl(hp, half, attu, 1)

        def attn_region(hp, half, fillers, every):
            attu = attu_p.tile([DH + 1, 4 * 512], F32, tag="attun", bufs=2, name="attu")
            attn_pass(hp, half, 0, attu, fillers, every)
            attn_pass(hp, half, 1, attu, fillers, every)
            while fillers:
                fillers.popleft()()
            normalize(hp, half, attu)

        # ---- schedule ----
        qk_group(0, 0)
        for st in range(4):
            v_group(st)
        qk_group(0, 1)
        for st in range(4, 8):
            v_group(st)

        # Fillers must WRITE data needed one region LATER (reads of data
        # produced by a filler inside its own consumer region would be
        # program-order read-before-write = stale SBUF).
        #   hp_i half-0 needs qt/kt[m_i] chunks 0,1 and v0..7 complete;
        #   hp_i half-1 needs qt/kt[m_i] chunks 2,3.
        attn_region(0, 0, deque(
            [lambda: qk_group(1, 0), lambda: qk_group(1, 1)]
            + [lambda st=st: v_group(st) for st in range(8, 12)]
        ), every=2)
        attn_region(1, 0, deque(
            [lambda: qk_group(2, 0), lambda: qk_group(2, 1),
             lambda: qk_group(0, 2), lambda: qk_group(0, 3)]
        ), every=4)
        attn_region(2, 0, deque(
            [lambda: qk_group(3, 0), lambda: qk_group(3, 1)]
        ), every=4)
        attn_region(3, 0, deque(
            [lambda st=st: v_group(st) for st in range(12, 16)]
        ), every=4)

        attn_region(0, 1, deque(
            [lambda: qk_group(1, 2), lambda: qk_group(1, 3),
             lambda: outproj_st(0), lambda: outproj_st(1)]
        ), every=7)
        attn_region(1, 1, deque(
            [lambda: qk_group(2, 2), lambda: qk_group(2, 3),
             lambda: outproj_st(2), lambda: outproj_st(3)]
        ), every=7)
        attn_region(2, 1, deque(
            [lambda: qk_group(3, 2), lambda: qk_group(3, 3),
             lambda: outproj_st(4), lambda: outproj_st(5)]
        ), every=7)
        attn_region(3, 1, deque(
            [lambda: outproj_st(6), lambda: outproj_st(7)]
        ), every=12)
        for st in range(8, ST):
            outproj_st(st)

    if split_waits:
        _split_sync_waits(nc)
    return nc


def get_program(mode, split_waits=True):
    key = (mode, split_waits)
    if key not in _PROGRAM_CACHE:
        _PROGRAM_CACHE[key] = build_program(mode, split_waits)
    return _PROGRAM_CACHE[key]


def _detect_mode(mask):
    m = np.asarray(mask)
    if np.array_equal(m != 0, np.tril(np.ones(m.shape, dtype=bool))):
        return "causal"
    if np.all(m != 0):
        return "full"
    raise NotImplementedError("only causal (tril) or all-ones masks supported")


def make_tri(mode):
    if mode != "causal":
        return np.zeros((P, P), dtype=np.float32)
    kk = np.arange(P)[:, None]
    cc = np.arange(P)[None, :]
    return np.where(cc >= kk, 0.0, NEG).astype(np.float32)


def make_in_maps(x, Wq, bq, Wk, Wv, bv, Wo, mode):
    import ml_dtypes

    bf = ml_dtypes.bfloat16
    x = np.asarray(x, dtype=np.float32)
    B = x.shape[0]
    tri = make_tri(mode)
    xts = [np.ascontiguousarray(x[b].T.astype(bf)) for b in range(B)]
    in_maps = []
    for c in range(NCORES):
        b, hg = divmod(c, 2)
        sl = slice(hg * EL, (hg + 1) * EL)
        in_maps.append(
            {
                "xt": xts[b],
                "wqt": np.ascontiguousarray(np.asarray(Wq[sl, :].T, np.float32).astype(bf)),
                "wkt": np.ascontiguousarray(np.asarray(Wk[sl, :].T, np.float32).astype(bf)),
                "wvt": np.ascontiguousarray(np.asarray(Wv[sl, :].T, np.float32).astype(bf)),
                "wot": np.ascontiguousarray(np.asarray(Wo[:, sl].T, np.float32).astype(bf)),
                "bq": np.ascontiguousarray(
                    np.asarray(bq[sl], np.float32).reshape(EL, 1)
                ),
                "bvrep": np.ascontiguousarray(
                    np.broadcast_to(np.asarray(bv[sl], np.float32), (P, EL))
                ),
                "tri": tri,
                "trib": np.ascontiguousarray((tri == 0.0).astype(bf)),
            }
        )
    return in_maps


def run(x, mask, Wq, bq, Wk, bk, Wv, bv, Wo, bo, trace=False, **spmd_kwargs):
    from concourse.bass_utils import run_bass_kernel_spmd

    mode = _detect_mode(mask)
    nc = get_program(mode)
    in_maps = make_in_maps(x, Wq, bq, Wk, Wv, bv, Wo, mode)

    res = run_bass_kernel_spmd(
        nc, in_maps, core_ids=list(range(NCORES)), trace=trace, **spmd_kwargs
    )
    B = np.asarray(x).shape[0]
    out = np.empty((B, S, D), dtype=np.float32)
    bo = np.asarray(bo, np.float32)
    for b in range(B):
        out[b] = res.results[2 * b]["out"] + res.results[2 * b + 1]["out"] + bo
    return out, res


def kernel(x, mask, Wq, bq, Wk, bk, Wv, bv, Wo, bo):
    out, _ = run(x, mask, Wq, bq, Wk, bk, Wv, bv, Wo, bo)
    return out
